# revision 2
# baseline (speedup 1.0000x reference)
"""Trainium2 Bass kernel for nn_LSTMClassifier (B=256,T=1024,D=64,H=128,C=10).

Data-parallel over batch across 8 cores (32 seqs/core), gate-major
layout (partitions = hidden units), per-core serial recurrence.

The per-step dependency cycle is PE(4 matmuls) -> Act(gates tanh) ->
DVE(cell stt ops) -> Act(tanh c) -> DVE(h) -> PE.  Optimizations:

1. All-tanh gates (sigmoid via tanh-half, scale folded into weights),
   one Act instruction for all 4 gates.  Doubled state C=2c, H=2h.
2. Persistent per-layer fp32 state tile [128, 8, 32] with block layout
   [ti tf tg to | c th u P] so u=(ti+1)*tg and P=(tf+1)*C fuse into ONE
   64-col scalar_tensor_tensor (in1 = strided blocks {tg, c}).
3. Bank-fill GEMMs (xg input transforms + L1 bias) are emitted as
   <=128-col pieces interleaved after each step's critical matmuls, so
   the in-order PE never head-of-line-blocks the recurrence on a long
   GEMM (PSUM zero-region: only the first piece of a bank carries
   start=True).
"""

import os
import sys
from collections import deque

import numpy as np

for _p in ("/opt/trn_rl_repo",):
    if _p not in sys.path:
        sys.path.insert(0, _p)

import ml_dtypes  # noqa: E402

B, T, D, H, C = 256, 1024, 64, 128, 10
NCORES, BL = 8, 32
# column-block order [i, f, g, o] == reference split order
PERM = [0, 1, 2, 3]
LAG = 2  # banks (of 4 steps) that L1 trails L0 in program order

_cache = {}


def _build_nc(t_steps):
    from contextlib import ExitStack

    import concourse.bass as bass
    import concourse.mybir as mybir
    from concourse import bacc
    from concourse.tile import TileContext

    dt = mybir.dt
    AF = mybir.ActivationFunctionType
    OP = mybir.AluOpType
    MS = bass.MemorySpace

    nc = bacc.Bacc(None, target_bir_lowering=False, debug=False)
    NB = t_steps // 4

    xta_d = nc.dram_tensor("xta", [D + 1, t_steps * BL], dt.bfloat16, kind="ExternalInput")
    w0aug_d = nc.dram_tensor("w0aug", [D + 1, 512], dt.bfloat16, kind="ExternalInput")
    whh0_d = nc.dram_tensor("whh0t", [H, 512], dt.bfloat16, kind="ExternalInput")
    w1_d = nc.dram_tensor("w1t", [H, 512], dt.bfloat16, kind="ExternalInput")
    whh1_d = nc.dram_tensor("whh1t", [H, 512], dt.bfloat16, kind="ExternalInput")
    b1_d = nc.dram_tensor("b1row", [4, H], dt.bfloat16, kind="ExternalInput")
    ind_d = nc.dram_tensor("ind", [4, 512], dt.bfloat16, kind="ExternalInput")
    whead_d = nc.dram_tensor("wheadt", [H, 16], dt.bfloat16, kind="ExternalInput")
    bhead_d = nc.dram_tensor("bhead", [16, 1], dt.float32, kind="ExternalInput")
    out_d = nc.dram_tensor("out", [16, BL], dt.float32, kind="ExternalOutput")

    with TileContext(nc) as tc, ExitStack() as ctx:
        consts = ctx.enter_context(tc.tile_pool(name="consts", bufs=1))
        xta = consts.tile([D + 1, t_steps * BL], dt.bfloat16, tag="xta")
        w0aug = consts.tile([D + 1, 512], dt.bfloat16, tag="w0aug")
        whh0 = consts.tile([H, 512], dt.bfloat16, tag="whh0")
        w1 = consts.tile([H, 512], dt.bfloat16, tag="w1")
        whh1 = consts.tile([H, 512], dt.bfloat16, tag="whh1")
        b1row = consts.tile([4, H], dt.bfloat16, tag="b1row")
        ind = consts.tile([4, 512], dt.bfloat16, tag="ind")
        wheadt = consts.tile([H, 16], dt.bfloat16, tag="wheadt")
        bhead = consts.tile([16, 1], dt.float32, tag="bhead")
        h1T = consts.tile([H, t_steps, BL], dt.bfloat16, tag="h1T")
        h2T = consts.tile([H, BL], dt.bfloat16, tag="h2T")
        hz = consts.tile([H, BL], dt.bfloat16, tag="hz")
        # persistent per-layer state: blocks [ti tf tg to | c th u P]
        st0 = consts.tile([H, 8, BL], dt.float32, tag="st0")
        st1 = consts.tile([H, 8, BL], dt.float32, tag="st1")
        outs = consts.tile([16, BL], dt.float32, tag="outs")

        # weights first (small, needed by the first GEMMs/steps), then the
        # xta stream in chunks, front-loaded small so slot 0 starts early
        for tl, dr in ((w0aug, w0aug_d), (whh0, whh0_d), (w1, w1_d), (whh1, whh1_d),
                       (b1row, b1_d), (ind, ind_d), (wheadt, whead_d), (bhead, bhead_d)):
            nc.sync.dma_start(tl[:], dr[:])
        bank_cols = 4 * BL
        cuts = [0, bank_cols, 4 * bank_cols, 16 * bank_cols]
        csz = (t_steps * BL - cuts[-1]) // 6
        cuts += [cuts[-1] + csz * i for i in range(1, 6)] + [t_steps * BL]
        for a, b in zip(cuts[:-1], cuts[1:]):
            if b > a:
                nc.sync.dma_start(xta[:, a:b], xta_d[:, a:b])
        nc.vector.memset(hz[:], 0.0)
        nc.vector.memset(st0[:, 4, :], 0.0)  # c = 0
        nc.vector.memset(st1[:, 4, :], 0.0)

        psum0 = ctx.enter_context(tc.tile_pool(name="psum0", bufs=3, space=MS.PSUM))
        psum1 = ctx.enter_context(tc.tile_pool(name="psum1", bufs=3, space=MS.PSUM))
        psumh = ctx.enter_context(tc.tile_pool(name="psumh", bufs=1, space=MS.PSUM))

        banks = [{}, {}]  # psum bank per layer, keyed by bank index
        pieces = deque()  # pending bank-fill matmul emitters (PE filler ops)

        # bank layout: col = j*128 + t_local*32 + b  (block-major so every
        # matmul output is a contiguous col range)
        def enqueue_l0(k):
            bank = psum0.tile([H, 512], dt.float32, tag="bank0")
            banks[0][k] = bank
            rhs = xta[:, 4 * k * BL:(4 * k + 4) * BL]
            for j in range(4):
                def p(j=j, bank=bank, rhs=rhs):
                    nc.tensor.matmul(bank[:, j * H:(j + 1) * H],
                                     w0aug[:, j * H:(j + 1) * H], rhs,
                                     start=(j == 0), stop=False)
                pieces.append(p)

        def enqueue_l1(k):
            bank = psum1.tile([H, 512], dt.float32, tag="bank1")
            banks[1][k] = bank
            rhs = h1T[:, 4 * k:4 * k + 4, :]
            # bias init via 4 indicator pieces; first carries start=True and
            # zeroes the whole bank (PSUM zero region == bank)
            for j in range(4):
                def pb(j=j, bank=bank):
                    nc.tensor.matmul(bank[:, j * H:(j + 1) * H], b1row[:],
                                     ind[:, j * H:(j + 1) * H],
                                     start=(j == 0), stop=False)
                pieces.append(pb)
            for j in range(4):
                def px(j=j, bank=bank, rhs=rhs):
                    nc.tensor.matmul(bank[:, j * H:(j + 1) * H],
                                     w1[:, j * H:(j + 1) * H], rhs,
                                     start=False, stop=False)
                pieces.append(px)

        def drain(n):
            for _ in range(min(n, len(pieces))):
                pieces.popleft()()

        def step(layer, t):
            tl = t % 4
            bank = banks[layer][t // 4]
            whh = whh0 if layer == 0 else whh1
            st = st0 if layer == 0 else st1
            if layer == 0:
                h_prev = hz if t == 0 else h1T[:, t - 1, :]
                h_out = h1T[:, t, :]
            else:
                h_prev = hz if t == 0 else h2T[:]
                h_out = h2T[:]
            base = tl * 32
            for j in range(4):
                nc.tensor.matmul(bank[:, j * H + base:j * H + base + 32],
                                 whh[:, j * H:(j + 1) * H], h_prev,
                                 start=False, stop=True)
            drain(2)  # bank-fill pieces ride behind the critical matmuls
            # gates: one tanh over all 4 blocks -> state blocks 0..3
            b4 = bank[:].rearrange("p (j x) -> p j x", j=4)
            nc.scalar.activation(st[:, 0:4, :], b4[:, :, base:base + 32], AF.Tanh)
            # fused u=(ti+1)*tg, P=(tf+1)*C : in1 = strided blocks {tg, c}
            tgc = st[:, 2:6, :].rearrange("p (a b) x -> p a b x", a=2)[:, :, 0, :]
            nc.vector.scalar_tensor_tensor(st[:, 6:8, :], st[:, 0:2, :], 1.0,
                                           tgc, OP.add, OP.mult)
            # C' = 0.5*P + u
            nc.vector.scalar_tensor_tensor(st[:, 4, :], st[:, 7, :], 0.5,
                                           st[:, 6, :], OP.mult, OP.add)
            # th = tanh(0.5*C')
            nc.scalar.activation(st[:, 5, :], st[:, 4, :], AF.Tanh, scale=0.5)
            # H' = (to+1)*th
            nc.vector.scalar_tensor_tensor(h_out, st[:, 3, :], 1.0,
                                           st[:, 5, :], OP.add, OP.mult)

        # prologue: L0 bank 0 filled directly
        enqueue_l0(0)
        drain(4)
        for k in range(NB + LAG):
            if k + 1 < NB:
                enqueue_l0(k + 1)
            kk = k - LAG
            if 0 <= kk + 1 < NB:
                enqueue_l1(kk + 1)
            if k < NB:
                for t in range(4 * k, 4 * k + 4):
                    step(0, t)
            if 0 <= kk < NB:
                for t in range(4 * kk, 4 * kk + 4):
                    step(1, t)
            drain(len(pieces) if k >= NB else 0)  # tail flush

        hp = psumh.tile([16, BL], dt.float32, tag="head")
        nc.tensor.matmul(hp[:], wheadt[:], h2T[:], start=True, stop=True)
        nc.scalar.activation(outs[:], hp[:], AF.Identity, bias=bhead[:, 0:1])
        nc.sync.dma_start(out_d[:], outs[:])

    nc.compile()
    return nc


def _pack_shared(W_ih0, W_hh0, b_ih0, b_hh0, W_ih1, W_hh1, b_ih1, b_hh1, W_head, b_head):
    bf16 = ml_dtypes.bfloat16
    b0 = (b_ih0 + b_hh0).astype(np.float32)
    b1 = (b_ih1 + b_hh1).astype(np.float32)

    # gate-block scale on the pre-activation: 0.5 for i/f/o (tanh-half
    # sigmoid), 1.0 for g.  Extra 0.5 on recurrent/L1-input weights since
    # the moving operand is H = 2h.
    gs = {0: 0.5, 1: 0.5, 2: 1.0, 3: 0.5}  # block order i,f,g,o

    w0aug = np.zeros((D + 1, 512), np.float32)
    whh0t = np.zeros((H, 512), np.float32)
    w1t = np.zeros((H, 512), np.float32)
    whh1t = np.zeros((H, 512), np.float32)
    b1row = np.zeros((4, H), np.float32)
    for j, g in enumerate(PERM):
        sl = slice(g * H, (g + 1) * H)
        s = gs[j]
        w0aug[:D, j * H:(j + 1) * H] = s * W_ih0[sl].T
        w0aug[D, j * H:(j + 1) * H] = s * b0[sl]
        whh0t[:, j * H:(j + 1) * H] = 0.5 * s * W_hh0[sl].T
        w1t[:, j * H:(j + 1) * H] = 0.5 * s * W_ih1[sl].T
        whh1t[:, j * H:(j + 1) * H] = 0.5 * s * W_hh1[sl].T
        b1row[j] = s * b1[sl]

    ind = np.zeros((4, 512), np.float32)
    cols = np.arange(512)
    for r in range(4):
        ind[r] = (cols // 128 == r).astype(np.float32)

    # head consumes H2 = 2h2
    wheadt = np.zeros((H, 16), np.float32)
    wheadt[:, :C] = 0.5 * W_head.T
    bhead = np.zeros((16, 1), np.float32)
    bhead[:C, 0] = b_head

    return {
        "w0aug": w0aug.astype(bf16), "whh0t": whh0t.astype(bf16),
        "w1t": w1t.astype(bf16), "whh1t": whh1t.astype(bf16),
        "b1row": b1row.astype(bf16), "ind": ind.astype(bf16),
        "wheadt": wheadt.astype(bf16), "bhead": bhead.astype(np.float32),
    }


def _make_xta(x_core, t_steps):
    # x_core [BL, T, D] -> [D+1, T*BL] with ones row (bias lane)
    bf16 = ml_dtypes.bfloat16
    xt = x_core[:, :t_steps, :].transpose(2, 1, 0).reshape(D, t_steps * BL)
    out = np.ones((D + 1, t_steps * BL), np.float32)
    out[:D] = xt
    return out.astype(bf16)


def run_cores(x, weights, t_steps=T, trace=False):
    from concourse.bass_utils import run_bass_kernel_spmd

    key = t_steps
    if key not in _cache:
        _cache[key] = _build_nc(t_steps)
    nc = _cache[key]

    shared = _pack_shared(**weights)
    in_maps = []
    for i in range(NCORES):
        m = dict(shared)
        m["xta"] = _make_xta(x[i * BL:(i + 1) * BL], t_steps)
        in_maps.append(m)
    res = run_bass_kernel_spmd(nc, in_maps, list(range(NCORES)), trace=trace)
    out = np.zeros((B, C), np.float32)
    for i in range(NCORES):
        out[i * BL:(i + 1) * BL] = res.results[i]["out"][:C, :].T
    return out, res


def kernel(x, W_ih0, W_hh0, b_ih0, b_hh0, W_ih1, W_hh1, b_ih1, b_hh1, W_head, b_head):
    weights = dict(W_ih0=W_ih0, W_hh0=W_hh0, b_ih0=b_ih0, b_hh0=b_hh0,
                   W_ih1=W_ih1, W_hh1=W_hh1, b_ih1=b_ih1, b_hh1=b_hh1,
                   W_head=W_head, b_head=b_head)
    weights = {k: np.asarray(v, np.float32) for k, v in weights.items()}
    out, _ = run_cores(np.asarray(x, np.float32), weights)
    return out
